# revision 2
# baseline (speedup 1.0000x reference)
"""Trainium2 Bass kernel for a dendritic layer:

    h = leaky(x @ Wd.T + bd)   # [B, 32768], Wd [32768, 1024]
    y = leaky(h @ Ws.T + bs)   # [B, 2048],  Ws [2048, 32768] block-diagonal

Sharding: tensor-parallel over the n_soma_connections axis. Core c owns
dendrites [c*4096, (c+1)*4096) == neurons [c*256, (c+1)*256), so the soma
stage is core-local (no cross-device reduction). The soma matmul collapses
to a per-column scale + segment-sum of 16 because Ws is block-diagonal.

Per core: one [256, 1024] @ [1024, 4096] GEMM on the tensor engine
(k-tiled into PSUM), bias fed into PSUM via a K=1 ones-row matmul, then a
DVE epilogue: leaky = max(0.1v, v), multiply by the flattened soma weights
(broadcast tile), segment-sum groups of 16, soma bias + leaky, DMA out.
"""

import os

import numpy as np

B = 256
IN_DIM = 1024
N_NEURONS = 2048
N_DENDRITES = 16
NSC = N_DENDRITES * N_NEURONS  # 32768
NCORES = 8
DSH = NSC // NCORES  # 4096 dendrites per core
NSH = N_NEURONS // NCORES  # 256 neurons per core
KT = IN_DIM // 128  # 8 k-tiles
NCH = DSH // 512  # 8 n-chunks of 512
BT = B // 128  # 2 batch tiles

# matmul dtype variant: "fp32" (exact, 4 cyc/row), "fp32r" (1 cyc/row),
# "bf16" (host-cast weights, halves HBM)
VARIANT = os.environ.get("DK_VARIANT", "fp32")


def _build_program(variant: str):
    import concourse.bacc as bacc
    import concourse.mybir as mybir
    import concourse.tile as tile

    f32 = mybir.dt.float32
    bf16 = mybir.dt.bfloat16
    mm_dt = {"fp32": f32, "fp32r": mybir.dt.float32r, "bf16": bf16}[variant]
    in_dt = bf16 if variant == "bf16" else f32
    add_op = mybir.AluOpType.add
    mult_op = mybir.AluOpType.mult
    max_op = mybir.AluOpType.max

    nc = bacc.Bacc("TRN2", target_bir_lowering=False, debug=False)

    xt_ap = nc.dram_tensor("xt", [128, KT, 128 * BT], in_dt, kind="ExternalInput").ap()
    wdt_ap = nc.dram_tensor(
        "wdt", [NCH, KT, 128, 512], in_dt, kind="ExternalInput"
    ).ap()
    bd_ap = nc.dram_tensor("bd", [1, DSH], f32, kind="ExternalInput").ap()
    wsb_ap = nc.dram_tensor("wsb", [128, DSH], f32, kind="ExternalInput").ap()
    bsb_ap = nc.dram_tensor("bsb", [128, NSH], f32, kind="ExternalInput").ap()
    y_ap = nc.dram_tensor("y", [B, NSH], f32, kind="ExternalOutput").ap()

    def mm_cast(ap):
        return ap.bitcast(mm_dt) if variant == "fp32r" else ap

    with tile.TileContext(nc) as tc:
        with (
            tc.tile_pool(name="const", bufs=1) as cpool,
            tc.tile_pool(name="xp", bufs=1) as xpool,
            tc.tile_pool(name="wp", bufs=3) as wpool,
            tc.tile_pool(name="ps", bufs=4, space="PSUM") as pspool,
            tc.tile_pool(name="hp", bufs=3) as hpool,
            tc.tile_pool(name="yp", bufs=1) as ypool,
        ):
            # constants / broadcast tiles
            ones_t = cpool.tile([1, 128], in_dt)
            nc.any.memset(ones_t[:], 1.0)
            bd_t = cpool.tile([1, DSH], f32)
            nc.sync.dma_start(bd_t[:], bd_ap[:])
            wsb_t = cpool.tile([128, DSH], f32)
            nc.sync.dma_start(wsb_t[:], wsb_ap[:])
            bsb_t = cpool.tile([128, NSH], f32)
            nc.sync.dma_start(bsb_t[:], bsb_ap[:])

            xt_t = xpool.tile([128, KT, 128 * BT], in_dt)
            nc.sync.dma_start(xt_t[:], xt_ap[:])

            ypre = []
            for b in range(BT):
                yt = ypool.tile([128, NSH], f32, tag=f"ypre{b}")
                ypre.append(yt)

            bd_mm = bd_t[:] if variant != "fp32r" else bd_t[:].bitcast(mm_dt)
            ones_mm = ones_t[:] if variant != "fp32r" else ones_t[:].bitcast(mm_dt)

            for nch in range(NCH):
                wt = wpool.tile([128, KT, 512], in_dt, tag="w")
                nc.sync.dma_start(
                    wt[:], wdt_ap[nch].rearrange("kt p n -> p kt n")
                )
                dsl = slice(nch * 512, (nch + 1) * 512)
                for b in range(BT):
                    ps = pspool.tile([128, 512], f32, tag="ps")
                    # bias row broadcast into all 128 partitions: K=1 matmul
                    nc.tensor.matmul(
                        ps[:],
                        lhsT=ones_mm,
                        rhs=bd_mm[:, dsl],
                        start=True,
                        stop=False,
                    )
                    for kt in range(KT):
                        nc.tensor.matmul(
                            ps[:],
                            lhsT=mm_cast(xt_t[:, kt, b * 128 : (b + 1) * 128]),
                            rhs=mm_cast(wt[:, kt, :]),
                            start=False,
                            stop=(kt == KT - 1),
                        )
                    # leaky relu on the Scalar engine (Prelu w/ alpha is
                    # exact on HW; single PSUM read allowed)
                    lh = hpool.tile([128, 512], f32, tag="lh")
                    nc.scalar.activation(
                        lh[:], ps[:], mybir.ActivationFunctionType.Prelu, alpha=0.1
                    )
                    # * soma weights (flattened block diag, broadcast tile)
                    tw = hpool.tile([128, 512], f32, tag="tw")
                    nc.vector.tensor_tensor(tw[:], lh[:], wsb_t[:, dsl], mult_op)
                    # segment-sum groups of 16 -> 32 neurons per chunk
                    nc.vector.tensor_reduce(
                        ypre[b][:, nch * 32 : (nch + 1) * 32],
                        tw[:].rearrange("p (g j) -> p g j", j=16),
                        axis=mybir.AxisListType.X,
                        op=add_op,
                    )

            for b in range(BT):
                ys = hpool.tile([128, NSH], f32, tag="ys")
                nc.vector.tensor_tensor(ys[:], ypre[b][:], bsb_t[:], add_op)
                yo = hpool.tile([128, NSH], f32, tag="yo")
                nc.vector.scalar_tensor_tensor(
                    yo[:], ys[:], 0.1, ys[:], mult_op, max_op
                )
                nc.sync.dma_start(y_ap[b * 128 : (b + 1) * 128, :], yo[:])

    nc.compile()
    return nc


def _prep_inputs(x, Wd, bd, Ws, bs, variant: str):
    """Build the per-core input maps (host-side shard + relayout)."""
    in_np = np.dtype(np.float32)
    if variant == "bf16":
        import ml_dtypes

        in_np = np.dtype(ml_dtypes.bfloat16)

    # xt[p, kt, m] = x[m, kt*128+p]
    xt = np.ascontiguousarray(
        x.T.reshape(KT, 128, B).transpose(1, 0, 2)
    ).astype(in_np)

    in_maps = []
    for c in range(NCORES):
        dsl = slice(c * DSH, (c + 1) * DSH)
        nsl = slice(c * NSH, (c + 1) * NSH)
        # wdt[nch, kt, p, n] = Wd[c*DSH + nch*512 + n, kt*128 + p]
        wdt = np.ascontiguousarray(
            Wd[dsl].T.reshape(KT, 128, NCH, 512).transpose(2, 0, 1, 3)
        ).astype(in_np)
        blk = Ws[nsl, dsl]  # [256, 4096] block diagonal
        ws_flat = blk.reshape(NSH, NSH, N_DENDRITES)[
            np.arange(NSH), np.arange(NSH), :
        ].reshape(1, DSH)
        in_maps.append(
            {
                "xt": xt,
                "wdt": wdt,
                "bd": np.ascontiguousarray(bd[dsl]).reshape(1, DSH),
                "wsb": np.ascontiguousarray(
                    np.broadcast_to(ws_flat, (128, DSH))
                ),
                "bsb": np.ascontiguousarray(
                    np.broadcast_to(bs[nsl].reshape(1, NSH), (128, NSH))
                ),
            }
        )
    return in_maps


_cache = {}


def run(x, Wd, bd, Ws, bs, variant=None, trace=False):
    from concourse.bass_utils import run_bass_kernel_spmd

    variant = variant or VARIANT
    if variant not in _cache:
        _cache[variant] = _build_program(variant)
    nc = _cache[variant]
    in_maps = _prep_inputs(x, Wd, bd, Ws, bs, variant)
    res = run_bass_kernel_spmd(nc, in_maps, list(range(NCORES)), trace=trace)
    y = np.concatenate([res.results[c]["y"] for c in range(NCORES)], axis=1)
    return y.astype(np.float32), res


def kernel(x, Wd, bd, Ws, bs):
    y, _ = run(x, Wd, bd, Ws, bs)
    return y


# revision 11
# speedup vs baseline: 2.1340x; 2.1340x over previous
"""Trainium2 Bass kernel for a dendritic layer:

    h = leaky(x @ Wd.T + bd)   # [B, 32768], Wd [32768, 1024]
    y = leaky(h @ Ws.T + bs)   # [B, 2048],  Ws [2048, 32768] block-diagonal

Sharding: tensor-parallel over the n_soma_connections axis. Core c owns
dendrites [c*4096, (c+1)*4096) == neurons [c*256, (c+1)*256), so the soma
stage is core-local (no cross-device reduction). The soma matmul collapses
to a per-column scale + segment-sum of 16 because Ws is block-diagonal.

Per core: one [256, 1024] @ [1024, 4096] GEMM on the tensor engine
(k-tiled into PSUM), bias fed into PSUM via a K=1 ones-row matmul, then a
DVE epilogue: leaky = max(0.1v, v), multiply by the flattened soma weights
(broadcast tile), segment-sum groups of 16, soma bias + leaky, DMA out.
"""

import os

import numpy as np

B = 256
IN_DIM = 1024
N_NEURONS = 2048
N_DENDRITES = 16
NSC = N_DENDRITES * N_NEURONS  # 32768
NCORES = 8
DSH = NSC // NCORES  # 4096 dendrites per core
NSH = N_NEURONS // NCORES  # 256 neurons per core
KT = IN_DIM // 128  # 8 k-tiles
NCH = DSH // 512  # 8 n-chunks of 512
BT = B // 128  # 2 batch tiles

# matmul dtype variant: "fp32" (exact, 4 cyc/row), "fp32r" (1 cyc/row),
# "bf16" (host-cast weights, halves HBM)
VARIANT = os.environ.get("DK_VARIANT", "fp32")


def _build_program(variant: str):
    import concourse.bacc as bacc
    import concourse.mybir as mybir
    import concourse.tile as tile

    f32 = mybir.dt.float32
    bf16 = mybir.dt.bfloat16
    # matmul-input dtype: fp32r tensors are declared fp32r end-to-end (DRAM
    # through SBUF) so walrus sees fp32r-typed producers.
    in_dt = {"fp32": f32, "fp32r": mybir.dt.float32r, "bf16": bf16}[variant]
    add_op = mybir.AluOpType.add
    mult_op = mybir.AluOpType.mult
    max_op = mybir.AluOpType.max

    nc = bacc.Bacc("TRN2", target_bir_lowering=False, debug=False)

    xt_ap = nc.dram_tensor("xt", [128, KT, 128 * BT], in_dt, kind="ExternalInput").ap()
    wdt_ap = nc.dram_tensor(
        "wdt", [NCH, KT, 128, 512], in_dt, kind="ExternalInput"
    ).ap()
    bd_ap = nc.dram_tensor("bd", [1, DSH], in_dt, kind="ExternalInput").ap()
    ones_ap = nc.dram_tensor("ones", [1, 128], in_dt, kind="ExternalInput").ap()
    wsb_ap = nc.dram_tensor("wsb", [128, DSH], f32, kind="ExternalInput").ap()
    bsb_ap = nc.dram_tensor("bsb", [128, NSH], f32, kind="ExternalInput").ap()
    y_ap = nc.dram_tensor("y", [B, NSH], f32, kind="ExternalOutput").ap()

    def mm_cast(ap):
        return ap

    with tile.TileContext(nc) as tc:
        with (
            tc.tile_pool(name="const", bufs=1) as cpool,
            tc.tile_pool(name="xp", bufs=1) as xpool,
            tc.tile_pool(name="wp", bufs=3) as wpool,
            tc.tile_pool(name="ps", bufs=4, space="PSUM") as pspool,
            tc.tile_pool(name="hp", bufs=3) as hpool,
            tc.tile_pool(name="yp", bufs=1) as ypool,
        ):
            # constants / broadcast tiles
            ones_t = cpool.tile([1, 128], in_dt)
            nc.sync.dma_start(ones_t[:], ones_ap[:])
            bd_t = cpool.tile([1, DSH], in_dt)
            nc.sync.dma_start(bd_t[:], bd_ap[:])
            wsb_t = cpool.tile([128, DSH], f32)
            nc.sync.dma_start(wsb_t[:], wsb_ap[:])
            bsb_t = cpool.tile([128, NSH], f32)
            nc.sync.dma_start(bsb_t[:], bsb_ap[:])

            xt_t = xpool.tile([128, KT, 128 * BT], in_dt)
            nc.sync.dma_start(xt_t[:], xt_ap[:])

            ypre = []
            for b in range(BT):
                yt = ypool.tile([128, NSH], f32, tag=f"ypre{b}")
                ypre.append(yt)

            bd_mm = bd_t[:]
            ones_mm = ones_t[:]

            for nch in range(NCH):
                wt = wpool.tile([128, KT, 512], in_dt, tag="w")
                nc.sync.dma_start(
                    wt[:], wdt_ap[nch].rearrange("kt p n -> p kt n")
                )
                dsl = slice(nch * 512, (nch + 1) * 512)
                for b in range(BT):
                    ps = pspool.tile([128, 512], f32, tag="ps")
                    # bias row broadcast into all 128 partitions: K=1 matmul
                    nc.tensor.matmul(
                        ps[:],
                        lhsT=ones_mm,
                        rhs=bd_mm[:, dsl],
                        start=True,
                        stop=False,
                    )
                    for kt in range(KT):
                        nc.tensor.matmul(
                            ps[:],
                            lhsT=mm_cast(xt_t[:, kt, b * 128 : (b + 1) * 128]),
                            rhs=mm_cast(wt[:, kt, :]),
                            start=False,
                            stop=(kt == KT - 1),
                        )
                    # leaky relu on the Scalar engine (Prelu w/ alpha is
                    # exact on HW; single PSUM read allowed)
                    lh = hpool.tile([128, 512], f32, tag="lh")
                    nc.scalar.activation(
                        lh[:], ps[:], mybir.ActivationFunctionType.Prelu, alpha=0.1
                    )
                    # * soma weights (flattened block diag, broadcast tile)
                    tw = hpool.tile([128, 512], f32, tag="tw")
                    nc.vector.tensor_tensor(tw[:], lh[:], wsb_t[:, dsl], mult_op)
                    # segment-sum groups of 16 -> 32 neurons per chunk
                    nc.vector.tensor_reduce(
                        ypre[b][:, nch * 32 : (nch + 1) * 32],
                        tw[:].rearrange("p (g j) -> p g j", j=16),
                        axis=mybir.AxisListType.X,
                        op=add_op,
                    )

            for b in range(BT):
                ys = hpool.tile([128, NSH], f32, tag="ys")
                nc.vector.tensor_tensor(ys[:], ypre[b][:], bsb_t[:], add_op)
                yo = hpool.tile([128, NSH], f32, tag="yo")
                nc.vector.scalar_tensor_tensor(
                    yo[:], ys[:], 0.1, ys[:], mult_op, max_op
                )
                nc.sync.dma_start(y_ap[b * 128 : (b + 1) * 128, :], yo[:])

    nc.compile()
    return nc


def _prep_inputs(x, Wd, bd, Ws, bs, variant: str):
    """Build the per-core input maps (host-side shard + relayout)."""
    in_np = np.dtype(np.float32)
    if variant == "bf16":
        import ml_dtypes

        in_np = np.dtype(ml_dtypes.bfloat16)

    # xt[p, kt, m] = x[m, kt*128+p]
    xt = np.ascontiguousarray(
        x.T.reshape(KT, 128, B).transpose(1, 0, 2)
    ).astype(in_np)

    in_maps = []
    for c in range(NCORES):
        dsl = slice(c * DSH, (c + 1) * DSH)
        nsl = slice(c * NSH, (c + 1) * NSH)
        # wdt[nch, kt, p, n] = Wd[c*DSH + nch*512 + n, kt*128 + p]
        wdt = np.ascontiguousarray(
            Wd[dsl].T.reshape(KT, 128, NCH, 512).transpose(2, 0, 1, 3)
        ).astype(in_np)
        blk = Ws[nsl, dsl]  # [256, 4096] block diagonal
        ws_flat = blk.reshape(NSH, NSH, N_DENDRITES)[
            np.arange(NSH), np.arange(NSH), :
        ].reshape(1, DSH)
        in_maps.append(
            {
                "xt": xt,
                "wdt": wdt,
                "ones": np.ones((1, 128), dtype=in_np),
                "bd": np.ascontiguousarray(bd[dsl]).reshape(1, DSH).astype(in_np),
                "wsb": np.ascontiguousarray(
                    np.broadcast_to(ws_flat, (128, DSH))
                ),
                "bsb": np.ascontiguousarray(
                    np.broadcast_to(bs[nsl].reshape(1, NSH), (128, NSH))
                ),
            }
        )
    return in_maps


_cache = {}


def run(x, Wd, bd, Ws, bs, variant=None, trace=False):
    from concourse.bass_utils import run_bass_kernel_spmd

    variant = variant or VARIANT
    if variant not in _cache:
        _cache[variant] = _build_program(variant)
    nc = _cache[variant]
    in_maps = _prep_inputs(x, Wd, bd, Ws, bs, variant)
    res = run_bass_kernel_spmd(nc, in_maps, list(range(NCORES)), trace=trace)
    y = np.concatenate([res.results[c]["y"] for c in range(NCORES)], axis=1)
    return y.astype(np.float32), res


def kernel(x, Wd, bd, Ws, bs):
    y, _ = run(x, Wd, bd, Ws, bs)
    return y
